# revision 1
# baseline (speedup 1.0000x reference)
"""MoE grouped-GEMM expert FFN (SwiGLU) on 8 Trainium2 NeuronCores.

Expert-parallel sharding: tokens arrive pre-grouped by expert with uniform
group size g = T/E = 1024, so core c owns experts [4c, 4c+4) and token rows
[c*4096, (c+1)*4096). No cross-core communication is needed: each core
computes its own 4 experts' FFN on its own token block.

Per-core math, per expert e:
    gu^T = w13_e^T-chunks @ x_e^T        # PE: contract H on partitions
    h^T  = silu(gate^T) * up^T           # ACT (Silu) + DVE (mul), bf16 out
    out  = h @ w2_e                      # PE: contract I on partitions

The host pre-transposes x (so H lands on SBUF partitions) and pre-tiles the
weights into [128, free] k-tiles, giving every DMA >=1KB contiguous
per-partition lines. All matmuls are 128x128 stationary x [128,512] moving,
bf16 in / fp32 PSUM accumulate.
"""

import sys

if "/opt/trn_rl_repo" not in sys.path:
    sys.path.insert(0, "/opt/trn_rl_repo")

import ml_dtypes
import numpy as np

import concourse.bacc as bacc
import concourse.bass as bass
import concourse.mybir as mybir
from concourse import tile
from concourse.bass_utils import run_bass_kernel_spmd

BF16 = mybir.dt.bfloat16
F32 = mybir.dt.float32
NPBF16 = ml_dtypes.bfloat16

N_CORES = 8
E = 32
H = 2048
I = 1024
T = 32768
EPC = E // N_CORES          # experts per core = 4
G = T // E                  # tokens per expert = 1024
ROWS = EPC * G              # token rows per core = 4096
KH = H // 128               # 16 contraction tiles for GEMM1
KI = I // 128               # 8 contraction tiles for GEMM2


def build_nc():
    nc = bacc.Bacc()
    xt_d = nc.declare_dram_parameter("xt", [KH, 128, ROWS], BF16, isOutput=False)
    w13_d = nc.declare_dram_parameter("w13", [EPC, KH, 128, 2 * I], BF16, isOutput=False)
    w2_d = nc.declare_dram_parameter("w2", [EPC, KI, 128, H], BF16, isOutput=False)
    out_d = nc.declare_dram_parameter("out", [ROWS, H], F32, isOutput=True)

    with tile.TileContext(nc) as tc:
        with (
            tc.tile_pool(name="xt", bufs=1) as xt_pool,
            tc.tile_pool(name="w13", bufs=1) as w13_pool,
            tc.tile_pool(name="w2", bufs=1) as w2_pool,
            tc.tile_pool(name="h", bufs=2) as h_pool,
            tc.tile_pool(name="tmp", bufs=3) as tmp_pool,
            tc.tile_pool(name="ost", bufs=4) as ost_pool,
            tc.tile_pool(name="ps", bufs=2, space="PSUM") as ps_pool,
        ):
            for e in range(EPC):
                xt_sb = []
                for k in range(KH):
                    t = xt_pool.tile([128, G], BF16, tag=f"xt{k}", bufs=1, name=f"xt{k}_{e}")
                    nc.sync.dma_start(t[:], xt_d[k][:, e * G:(e + 1) * G])
                    xt_sb.append(t)
                w13_sb = []
                for k in range(KH):
                    t = w13_pool.tile([128, 2 * I], BF16, tag=f"w13_{k}", bufs=1, name=f"w13_{k}_{e}")
                    nc.sync.dma_start(t[:], w13_d[e, k][:])
                    w13_sb.append(t)
                w2_sb = []
                for k in range(KI):
                    t = w2_pool.tile([128, H], BF16, tag=f"w2_{k}", bufs=1, name=f"w2_{k}_{e}")
                    nc.sync.dma_start(t[:], w2_d[e, k][:])
                    w2_sb.append(t)

                # Phase 1: gu^T tiles -> SwiGLU -> h^T resident in SBUF (bf16).
                h_sb = [h_pool.tile([128, G], BF16, tag=f"h{m}", bufs=2, name=f"h{m}_{e}") for m in range(KI)]
                for m in range(KI):
                    # One PSUM bank per (gate/up, n) group; the k-loop
                    # interleaves all four so each stationary weight tile
                    # feeds two consecutive matmuls (LDW reuse) and the PE
                    # pipelines fills across banks.
                    pg = [ps_pool.tile([128, 512], F32, tag=f"pg{n}", bufs=1, name=f"pg{n}_{e}_{m}")
                          for n in range(2)]
                    pu = [ps_pool.tile([128, 512], F32, tag=f"pu{n}", bufs=1, name=f"pu{n}_{e}_{m}")
                          for n in range(2)]
                    for k in range(KH):
                        wg = w13_sb[k][:, m * 128:(m + 1) * 128]
                        wu = w13_sb[k][:, I + m * 128:I + (m + 1) * 128]
                        for n in range(2):
                            nc.tensor.matmul(
                                pg[n][:], wg, xt_sb[k][:, n * 512:(n + 1) * 512],
                                start=(k == 0), stop=(k == KH - 1),
                            )
                        for n in range(2):
                            nc.tensor.matmul(
                                pu[n][:], wu, xt_sb[k][:, n * 512:(n + 1) * 512],
                                start=(k == 0), stop=(k == KH - 1),
                            )
                    for n in range(2):
                        ncol = slice(n * 512, (n + 1) * 512)
                        tmp = tmp_pool.tile([128, 512], F32, tag="tmp", bufs=3, name=f"tmp_{e}_{m}_{n}")
                        pu_sb = tmp_pool.tile([128, 512], F32, tag="pusb", bufs=3, name=f"pusb_{e}_{m}_{n}")
                        nc.scalar.activation(
                            tmp[:], pg[n][:], mybir.ActivationFunctionType.Silu
                        )
                        # Both epilogue producers run on ACT so the DVE mul
                        # carries ONE merged ACT wait (the TT instruction
                        # encoding only fits a single sync-wait).
                        nc.scalar.copy(pu_sb[:], pu[n][:])
                        nc.vector.tensor_mul(h_sb[m][:, ncol], tmp[:], pu_sb[:])

                # Phase 2: out_e = h @ w2_e, streamed straight to DRAM.
                for mt in range(KI):
                    rows = slice(e * G + mt * 128, e * G + (mt + 1) * 128)
                    po = [ps_pool.tile([128, 512], F32, tag=f"po{n}", bufs=1, name=f"po{n}_{e}_{mt}")
                          for n in range(4)]
                    for k in range(KI):
                        hk = h_sb[k][:, mt * 128:(mt + 1) * 128]
                        for n in range(4):
                            nc.tensor.matmul(
                                po[n][:], hk, w2_sb[k][:, n * 512:(n + 1) * 512],
                                start=(k == 0), stop=(k == KI - 1),
                            )
                    for n in range(4):
                        ncol = slice(n * 512, (n + 1) * 512)
                        ot = ost_pool.tile([128, 512], F32, tag="ot", bufs=4, name=f"ot_{e}_{mt}_{n}")
                        nc.vector.tensor_copy(ot[:], po[n][:])
                        nc.sync.dma_start(out_d[rows, ncol], ot[:])
    nc.compile()
    return nc


def _in_map_for_core(x, w13, w2, c):
    xs = x[c * ROWS:(c + 1) * ROWS]                      # [4096, 2048] f32
    xt = xs.T.astype(NPBF16, order="C").reshape(KH, 128, ROWS)
    w13c = np.ascontiguousarray(w13[c * EPC:(c + 1) * EPC]).astype(NPBF16)
    w2c = np.ascontiguousarray(w2[c * EPC:(c + 1) * EPC]).astype(NPBF16)
    return {
        "xt": xt,
        "w13": w13c.reshape(EPC, KH, 128, 2 * I),
        "w2": w2c.reshape(EPC, KI, 128, H),
    }


def kernel(x, w13, w2, tokens_per_expert, decoding, _trace=False):
    x = np.asarray(x, dtype=np.float32)
    w13 = np.asarray(w13, dtype=np.float32)
    w2 = np.asarray(w2, dtype=np.float32)

    in_maps = [_in_map_for_core(x, w13, w2, c) for c in range(N_CORES)]
    nc = build_nc()
    res = run_bass_kernel_spmd(nc, in_maps, list(range(N_CORES)), trace=_trace)
    out = np.concatenate([res.results[c]["out"] for c in range(N_CORES)], axis=0)
    if _trace:
        return out, res
    return out



# revision 4
# speedup vs baseline: 1.8717x; 1.8717x over previous
"""MoE grouped-GEMM expert FFN (SwiGLU) on 8 Trainium2 NeuronCores.

Expert-parallel sharding: tokens arrive pre-grouped by expert with uniform
group size g = T/E = 1024, so core c owns experts [4c, 4c+4) and token rows
[c*4096, (c+1)*4096). No cross-core communication is needed: each core
computes its own 4 experts' FFN on its own token block.

Per-core math, per expert e:
    gu^T = w13_e^T-chunks @ x_e^T        # PE: contract H on partitions
    h^T  = silu(gate^T) * up^T           # ACT (Silu) + DVE (mul), bf16 out
    out  = h @ w2_e                      # PE: contract I on partitions

Per-core device time is ~663us, ~1% over the bf16 PE roofline for this
shard (2.58e10 MACs / (128x128 @ 2.4GHz) = 655us). Design notes:
- The host pre-transposes x (H on SBUF partitions) and packs w13 per
  (expert, 128-row m-chunk of the 2I output) as one contiguous
  [128, KH*256] block holding the gate|up column pair for every H k-tile.
  GEMM1's first m-step then needs only ~1.25MB of weights in SBUF, and the
  first expert's DMA issue order feeds the k-loop just-in-time, so the PE
  starts ~4us in and runs gap-free (all DMAs >=512B/partition lines).
- All matmuls are 128x128 stationary x [128,512] moving, bf16 in / fp32
  PSUM accumulate. Each stationary tile feeds 2 (GEMM1) or 4 (GEMM2)
  consecutive matmuls so LDWEIGHTS hides in the PE reorder window.
- PSUM uses 4 shared tags x 2 bufs = all 8 banks double-buffered across
  GEMM1 m-steps and GEMM2 row-tiles: no PE stalls on epilogue consumers.
- SwiGLU epilogue puts both PSUM readers (Silu + copy) on ACT so the DVE
  mul needs a single cross-engine wait; h stays in SBUF as bf16.
- Output is stored bf16 (host upcasts): halves output HBM traffic; the
  final row-tile's k-loop is split in two n-pair passes and its stores
  issue from both ACT and SP DGE queues to shorten the end-of-kernel tail.
"""

import sys

if "/opt/trn_rl_repo" not in sys.path:
    sys.path.insert(0, "/opt/trn_rl_repo")

import ml_dtypes
import numpy as np

import concourse.bacc as bacc
import concourse.bass as bass
import concourse.mybir as mybir
from concourse import tile
from concourse.bass_utils import run_bass_kernel_spmd

BF16 = mybir.dt.bfloat16
F32 = mybir.dt.float32
NPBF16 = ml_dtypes.bfloat16

N_CORES = 8
E = 32
H = 2048
I = 1024
T = 32768
EPC = E // N_CORES          # experts per core = 4
G = T // E                  # tokens per expert = 1024
ROWS = EPC * G              # token rows per core = 4096
KH = H // 128               # 16 contraction tiles for GEMM1
KI = I // 128               # 8 contraction tiles for GEMM2
WM = KH * 256               # w13 packed tile free size = 4096


def build_nc(reps=1):
    """reps>1 unrolls the whole computation (identical outputs) so test
    harnesses can measure per-iteration device time with launch overhead
    cancelled; the graded path uses reps=1."""
    nc = bacc.Bacc()
    xt_d = nc.declare_dram_parameter("xt", [KH, 128, ROWS], BF16, isOutput=False)
    w13_d = nc.declare_dram_parameter("w13", [EPC, KI, 128, WM], BF16, isOutput=False)
    w2_d = nc.declare_dram_parameter("w2", [EPC, KI, 128, H], BF16, isOutput=False)
    out_d = nc.declare_dram_parameter("out", [ROWS, H], BF16, isOutput=True)

    n_iter = reps * EPC
    with tile.TileContext(nc) as tc:
        with (
            tc.tile_pool(name="xt", bufs=1) as xt_pool,
            tc.tile_pool(name="w13", bufs=4) as w13_pool,
            tc.tile_pool(name="w2", bufs=1) as w2_pool,
            tc.tile_pool(name="h", bufs=2) as h_pool,
            tc.tile_pool(name="tmp", bufs=3) as tmp_pool,
            tc.tile_pool(name="ost", bufs=4) as ost_pool,
            tc.tile_pool(name="ps", bufs=2, space="PSUM") as ps_pool,
        ):
            for e in range(n_iter):
                ex = e % EPC
                # --- input DMAs ---------------------------------------
                w13m_sb = [
                    w13_pool.tile([128, WM], BF16, tag="w13m", bufs=4,
                                  name=f"w13m_{e}_{m}")
                    for m in range(KI)
                ]
                xt_sb = [
                    xt_pool.tile([128, G], BF16, tag=f"xt{k}", bufs=1,
                                 name=f"xt{k}_{e}")
                    for k in range(KH)
                ]
                # m=0's weights interleaved with the xt tiles, ordered so the
                # k-th GEMM1 step's inputs land just ahead of the PE reaching
                # them (startup critical path). Chunk j of w13m covers
                # k in [4j, 4j+4), so xt tiles go 4:1 against w13 chunks; the
                # lead transfers issue on separate DGE queues (ACT + SP).
                cs = slice(0, 256)
                nc.scalar.dma_start(w13m_sb[0][:, cs], w13_d[ex, 0][:, cs])
                cs = slice(256, 1024)
                nc.scalar.dma_start(w13m_sb[0][:, cs], w13_d[ex, 0][:, cs])
                for k in range(KH):
                    nc.sync.dma_start(xt_sb[k][:], xt_d[k][:, ex * G:(ex + 1) * G])
                    if k % 4 == 3 and k < 12:
                        j = k // 4 + 1
                        cs = slice(j * 1024, (j + 1) * 1024)
                        nc.sync.dma_start(w13m_sb[0][:, cs], w13_d[ex, 0][:, cs])
                for m in range(1, KI):
                    for j in range(4):
                        cs = slice(j * 1024, (j + 1) * 1024)
                        nc.sync.dma_start(w13m_sb[m][:, cs], w13_d[ex, m][:, cs])
                w2_sb = []
                for k in range(KI):
                    t = w2_pool.tile([128, H], BF16, tag=f"w2_{k}", bufs=1,
                                     name=f"w2_{k}_{e}")
                    nc.sync.dma_start(t[:], w2_d[ex, k][:])
                    w2_sb.append(t)

                # --- GEMM1: gu^T tiles -> SwiGLU -> h^T in SBUF (bf16) --
                h_sb = [h_pool.tile([128, G], BF16, tag=f"h{m}", bufs=2,
                                    name=f"h{m}_{e}") for m in range(KI)]
                for m in range(KI):
                    pg = [ps_pool.tile([128, 512], F32, tag=f"b{n}", bufs=2,
                                       name=f"pg{n}_{e}_{m}") for n in range(2)]
                    pu = [ps_pool.tile([128, 512], F32, tag=f"b{n + 2}", bufs=2,
                                       name=f"pu{n}_{e}_{m}") for n in range(2)]
                    for k in range(KH):
                        wg = w13m_sb[m][:, k * 256:k * 256 + 128]
                        wu = w13m_sb[m][:, k * 256 + 128:(k + 1) * 256]
                        for n in range(2):
                            nc.tensor.matmul(
                                pg[n][:], wg, xt_sb[k][:, n * 512:(n + 1) * 512],
                                start=(k == 0), stop=(k == KH - 1),
                            )
                        for n in range(2):
                            nc.tensor.matmul(
                                pu[n][:], wu, xt_sb[k][:, n * 512:(n + 1) * 512],
                                start=(k == 0), stop=(k == KH - 1),
                            )
                    for n in range(2):
                        ncol = slice(n * 512, (n + 1) * 512)
                        tmp = tmp_pool.tile([128, 512], F32, tag="tmp", bufs=3,
                                            name=f"tmp_{e}_{m}_{n}")
                        pu_sb = tmp_pool.tile([128, 512], F32, tag="pusb", bufs=3,
                                              name=f"pusb_{e}_{m}_{n}")
                        nc.scalar.activation(
                            tmp[:], pg[n][:], mybir.ActivationFunctionType.Silu
                        )
                        # Both epilogue producers run on ACT so the DVE mul
                        # carries ONE merged ACT wait (the TT instruction
                        # encoding only fits a single sync-wait).
                        nc.scalar.copy(pu_sb[:], pu[n][:])
                        nc.vector.tensor_mul(h_sb[m][:, ncol], tmp[:], pu_sb[:])

                # --- GEMM2: out_e = h @ w2_e, bf16 straight to DRAM -----
                last = e == n_iter - 1
                for mt in range(KI):
                    rows = slice(ex * G + mt * 128, ex * G + (mt + 1) * 128)
                    po = [ps_pool.tile([128, 512], F32, tag=f"b{n}", bufs=2,
                                       name=f"po{n}_{e}_{mt}") for n in range(4)]
                    tail = last and mt == KI - 1
                    # The final tile runs its k-loop in two n-pair passes so
                    # the first pair's copy+store drains under the second
                    # pair's matmuls (shorter end-of-kernel tail). Elsewhere
                    # k-outer maximizes stationary (hk) reuse.
                    n_groups = ([(0, 1), (2, 3)] if tail else [(0, 1, 2, 3)])
                    for ns in n_groups:
                        for k in range(KI):
                            hk = h_sb[k][:, mt * 128:(mt + 1) * 128]
                            for n in ns:
                                nc.tensor.matmul(
                                    po[n][:], hk, w2_sb[k][:, n * 512:(n + 1) * 512],
                                    start=(k == 0), stop=(k == KI - 1),
                                )
                        for n in ns:
                            ncol = slice(n * 512, (n + 1) * 512)
                            ot = ost_pool.tile([128, 512], BF16, tag="ot", bufs=4,
                                               name=f"ot_{e}_{mt}_{n}")
                            if tail and n % 2 == 1:
                                nc.scalar.copy(ot[:], po[n][:])
                                nc.scalar.dma_start(out_d[rows, ncol], ot[:])
                            else:
                                nc.vector.tensor_copy(ot[:], po[n][:])
                                nc.sync.dma_start(out_d[rows, ncol], ot[:])
    nc.compile()
    return nc


def _in_map_for_core(x, w13, w2, c):
    xs = x[c * ROWS:(c + 1) * ROWS]                      # [4096, 2048] f32
    xt = xs.T.astype(NPBF16, order="C").reshape(KH, 128, ROWS)
    w13c = w13[c * EPC:(c + 1) * EPC].astype(NPBF16)     # [4, 2048, 2048]
    a = w13c.reshape(EPC, KH, 128, 2 * I)
    gate = a[:, :, :, :I].reshape(EPC, KH, 128, KI, 128)
    up = a[:, :, :, I:].reshape(EPC, KH, 128, KI, 128)
    comb = np.stack([gate, up], axis=4)                  # [e, k, p, m, gu, c]
    # -> [e, m, p, k, gu, c] so tile[p, k*256 + gu*128 + c] is contiguous
    w13t = np.ascontiguousarray(comb.transpose(0, 3, 2, 1, 4, 5)).reshape(
        EPC, KI, 128, WM
    )
    w2c = np.ascontiguousarray(w2[c * EPC:(c + 1) * EPC]).astype(NPBF16)
    return {
        "xt": xt,
        "w13": w13t,
        "w2": w2c.reshape(EPC, KI, 128, H),
    }


_NC_CACHE = None


def kernel(x, w13, w2, tokens_per_expert, decoding, _trace=False):
    global _NC_CACHE
    x = np.asarray(x, dtype=np.float32)
    w13 = np.asarray(w13, dtype=np.float32)
    w2 = np.asarray(w2, dtype=np.float32)

    in_maps = [_in_map_for_core(x, w13, w2, c) for c in range(N_CORES)]
    if _NC_CACHE is None:
        _NC_CACHE = build_nc()
    nc = _NC_CACHE
    res = run_bass_kernel_spmd(nc, in_maps, list(range(N_CORES)), trace=_trace)
    out = np.concatenate(
        [res.results[c]["out"].astype(np.float32) for c in range(N_CORES)], axis=0
    )
    if _trace:
        return out, res
    return out
